# revision 84
# baseline (speedup 1.0000x reference)
"""MoE (MiniMax decoder MLP) Trainium2 kernel — expert-parallel across 8 NeuronCores.

Strategy (per the expert-parallel sharding hint):
  - Host computes the router (softmax + top-2 + renormalize) — this IS the
    sharding decision — and dispatches each token's activation row to the
    core(s) owning its selected expert(s).
  - Core e holds expert e's weights and computes silu(x @ Wg) * (x @ Wu) @ Wd
    for its routed tokens (padded to a common capacity C), scaling the output
    by the renormalized combine weight on-device.
  - Host scatter-adds the per-expert outputs back into the full [T, H] output.

Compute is bf16 on the TensorEngine (fp32 PSUM accumulation).

Layout/pipeline choices (v2):
  - Token windows are an even split of C into ceil(C/512) parts, so every
    window is ~420 wide: each matmul moves >=165ns of rows, comfortably above
    the ~109ns small-matmul floor (the old 51-token tail window paid
    352*109ns for 21ns of rows).
  - The down-projection runs token-moving: stationary wd tile [128 i, 128 h],
    moving gated [128 i, W tokens] -> psum [128 h, W]. Cost is proportional
    to actual tokens — no 128-token m-tile quantization (saves a full
    m-tile sweep per core vs. the token-stationary form).
  - Output is written transposed (out_t [H, C]); the host untransposes
    during the combine. Combine weights are applied on-device via a
    host-replicated [128, C] broadcast tile (per-token = per-column in this
    orientation, so a tensor_tensor multiply, not a per-partition scalar).
  - wd is staged h-major so m2's first strip only needs a 0.36MB chunk.
  - All weight/activation DMAs are large per-partition-contiguous transfers.
    Transfers stripe across all 16 HW DMA engines, but queue ISSUE order is
    what matters at startup: window-0 x rides the scalar queue in k-chunks
    (so the first gate matmuls can start per-chunk via region-level
    hazards), weights/cwb/window-1 serialize on sync behind the critical
    stream, and windows 2+ sit on gpsimd where the xpool buffer hazard
    gates them (an eagerly-issued gpsimd DMA would flood the engine
    descriptor FIFOs ahead of the startup stream — measured +6.5us).
  - Warm-up matmuls on garbage data run before and interleaved with the
    DMA-paced first i-sweep: any PE idle gap resets the clock ramp and the
    next ~3us of matmuls run 2-3x slow, so the fillers keep the array busy
    through the startup DMA tail at zero real cost.
"""

import os
import sys

import numpy as np

_EXTRA_PATHS = [
    "/root/.axon_site",
    "/root/.axon_site/_ro/trn_rl_repo",
    "/root/.axon_site/_ro/pypackages",
    "/opt/trn_rl_repo",
    "/opt/pypackages",
]
try:
    import concourse.bass  # noqa: F401
except ImportError:  # pragma: no cover
    sys.path[:0] = [p for p in _EXTRA_PATHS if p not in sys.path]

import ml_dtypes

B, S, H = 4, 2048, 2048
I = 1408  # expert intermediate size
E = 8  # num experts
K = 2  # experts per token
N_CORES = 8

KT = H // 128  # 16 contraction tiles over H
IT = I // 128  # 11 tiles over I
HT = H // 128  # 16 output h-tiles (down-proj)
WBLK = KT * 128  # free-dim span of one i-block in the wg/wu SBUF image

_NC_CACHE = {}


def _windows(C):
    # Even split into ceil(C/512) windows (a matmul's output free size is
    # hard-capped at 512 fp32 by the ISA — s3d3_mm_num_elements — so wider
    # windows spanning two PSUM banks do not compile). Even sizes keep
    # every window well above the ~260-token width where per-matmul time
    # stops being row-proportional (LDWEIGHTS floor).
    nw = -(-C // 512)
    base, extra = divmod(C, nw)
    ws = []
    o = 0
    for wi in range(nw):
        w = base + (1 if wi < extra else 0)
        ws.append((o, w))
        o += w
    return ws


def _build_nc(C):
    """Build + compile the per-core expert MLP program for capacity C tokens."""
    import concourse.mybir as mybir
    import concourse.tile as tile
    from concourse import bacc

    fp32 = mybir.dt.float32
    bf16 = mybir.dt.bfloat16
    mult = mybir.AluOpType.mult
    silu_fn = mybir.ActivationFunctionType.Silu

    windows = _windows(C)
    WMAX = max(w for _, w in windows)

    nc = bacc.Bacc("TRN2", target_bir_lowering=False, debug=False, num_devices=N_CORES)

    # All inputs pre-swizzled to SBUF-image layouts (see kernel() below).
    xt = nc.dram_tensor("xt", [128, KT * C], bf16, kind="ExternalInput")
    wg = nc.dram_tensor("wg", [128, IT * WBLK], bf16, kind="ExternalInput")
    wu = nc.dram_tensor("wu", [128, IT * WBLK], bf16, kind="ExternalInput")
    wd = nc.dram_tensor("wd", [128, HT * IT * 128], bf16, kind="ExternalInput")
    cwb = nc.dram_tensor("cwb", [128, C], fp32, kind="ExternalInput")
    # bf16 output: each expert contribution is rounded to bf16 before the
    # host combine (~0.3% of element scale, negligible next to the bf16
    # matmul error) — halves all output DMA traffic and the final-strip
    # transfer that gates the fixed end-of-kernel semaphore sweep.
    out_t = nc.dram_tensor("out_t", [H, C], bf16, kind="ExternalOutput")

    with tile.TileContext(nc) as tc:
        with (
            tc.tile_pool(name="wpool", bufs=1) as wpool,
            tc.tile_pool(name="xpool", bufs=2) as xpool,
            tc.tile_pool(name="gpool", bufs=2) as gpool,
            tc.tile_pool(name="spool", bufs=2) as spool,
            tc.tile_pool(name="opool", bufs=3) as opool,
            tc.tile_pool(name="cwpool", bufs=1) as cwpool,
            tc.tile_pool(name="warm", bufs=1) as warm,
            tc.tile_pool(name="pgp", bufs=2, space="PSUM") as pgp,
            tc.tile_pool(name="pup", bufs=2, space="PSUM") as pup,
            tc.tile_pool(name="pop", bufs=3, space="PSUM") as pop,
            tc.tile_pool(name="pwp", bufs=1, space="PSUM") as pwp,
        ):
            # PE warm-up on garbage SBUF data: ramps the PE clock gate while
            # the first DMAs are still in flight. Never read back. More
            # warm matmuls run as fillers inside the first i-sweep (see
            # emit_matmul1) to keep the ramp from resetting.
            # Vector memset feeds the warmup earliest (a gpsimd DMA-fed
            # variant measured firstPE 9.7us vs 8.0us for this path).
            wsrc = warm.tile([128, 512], bf16, name="wsrc", tag="wsrc")
            nc.vector.memset(wsrc[:], 1.0)
            pw = pwp.tile([128, 512], fp32, name="pw", tag="pw")

            def warm_mm(width=512):
                nc.tensor.matmul(
                    pw[:, :width], wsrc[:, :128], wsrc[:, :width],
                    start=True, stop=True,
                )

            # Bridge PE-availability (~7.4us) past the x-w0 chunk-2/3
            # arrival (~13.3us). The PE stream STALLS at the first
            # unsatisfied matmul regardless of later fillers being
            # dependency-free (in-order sequencer); an 8-matmul chain left a
            # measured 2us stall at k2-3 plus a ~4us mid-pstate ramp
            # recovery. 12 matmuls carry the ramp past the whole window-0
            # chunk stream.
            for _ in range(12):
                warm_mm()

            def dma_xt_window(o, W, ksplits, engines):
                # ksplits: k-tiles per chunk; engines: issue queue per chunk.
                t = xpool.tile([128, KT * WMAX], bf16, name="xt_sb", tag="xt_sb")
                s0 = 0
                for nk, eng in zip(ksplits, engines):
                    s1 = s0 + nk * W
                    eng.dma_start(t[:, s0:s1], xt.ap()[:, KT * o + s0 : KT * o + s1])
                    s0 = s1
                return t

            # Window-0 activations: fine-grained k-chunks so the first gate
            # matmuls ride the DMA tail (each matmul k depends only on its
            # own chunk via region-level hazards). All on the scalar HWDGE
            # queue: its 667ns-per-DMA issue rate paces the descriptors, and
            # keeping the gpsimd queue EMPTY at startup matters — its 25ns
            # issue rate would dump megabytes of non-critical descriptors
            # into the engine FIFOs ahead of these (descriptor-FIFO priority
            # inversion, measured +6.5us).
            xt0_sb = dma_xt_window(
                *windows[0],
                ksplits=[1, 1, 2, 4, 4, 4],
                engines=[nc.scalar, nc.scalar, nc.gpsimd, nc.gpsimd,
                         nc.scalar, nc.scalar],
            )
            cwb_sb = cwpool.tile([128, C], fp32, name="cwb_sb", tag="cwb_sb")

            # Expert weights (bf16) on the sync queue: interleaved gate/up
            # i-blocks (0.5MB each, demand order of matmul1), then the
            # h-major down-proj chunks (0.36MB each, demand order of matmul2).
            wg_sb = wpool.tile([128, IT * WBLK], bf16, name="wg_sb", tag="wg_sb")
            wu_sb = wpool.tile([128, IT * WBLK], bf16, name="wu_sb", tag="wu_sb")
            wd_sb = wpool.tile([128, HT * IT * 128], bf16, name="wd_sb", tag="wd_sb")
            for i in range(IT):
                # The i=0 blocks split into k-chunks: the very first LDWEIGHTS
                # only needs wg's first [128,128] k-tile, so a 0.125MB chunk
                # (lands ~2us before the whole 0.5MB block would) starts the
                # first real matmul that much earlier via region hazards.
                nks = [2, 2, 4, 8] if i == 0 else [KT]
                s0 = 0
                for nk in nks:
                    s1 = s0 + nk * 128
                    nc.sync.dma_start(
                        wg_sb[:, i * WBLK + s0 : i * WBLK + s1],
                        wg.ap()[:, i * WBLK + s0 : i * WBLK + s1],
                    )
                    s0 = s1
                s0 = 0
                for nk in nks:
                    s1 = s0 + nk * 128
                    nc.sync.dma_start(
                        wu_sb[:, i * WBLK + s0 : i * WBLK + s1],
                        wu.ap()[:, i * WBLK + s0 : i * WBLK + s1],
                    )
                    s0 = s1
            WDB = IT * 128  # columns per h-chunk of the wd image
            for h in range(0, HT, 2):
                nc.sync.dma_start(
                    wd_sb[:, h * WDB : (h + 2) * WDB],
                    wd.ap()[:, h * WDB : (h + 2) * WDB],
                )
            # Combine weights (needed ~140us in) ride the sync queue AFTER
            # the weights so they can't contend with the startup stream.
            nc.sync.dma_start(cwb_sb[:], cwb.ap()[:])

            def emit_matmul1(xt_sb, W, fillers=False):
                """silu(x@Wg) * (x@Wu) for one token window -> gated^T tiles.

                fillers: interleave dependency-free warm matmuls between the
                first i-block's k-steps. During startup those k-steps are
                paced by the activation chunk DMAs; a bare pipeline would
                idle ~0.5us per chunk and each idle resets the PE clock ramp
                (the next real matmul then runs 2-3x slow). The fillers keep
                the PE array busy through the DMA tail at zero real cost.
                """
                gated = []
                for i in range(IT):
                    pg = pgp.tile([128, 512], fp32, name="pg", tag="pg")
                    pu = pup.tile([128, 512], fp32, name="pu", tag="pu")
                    for k in range(KT):
                        if fillers and i == 0:
                            warm_mm()
                        nc.tensor.matmul(
                            pg[:, :W],
                            wg_sb[:, i * WBLK + k * 128 : i * WBLK + (k + 1) * 128],
                            xt_sb[:, k * W : (k + 1) * W],
                            start=(k == 0),
                            stop=(k == KT - 1),
                        )
                    for k in range(KT):
                        nc.tensor.matmul(
                            pu[:, :W],
                            wu_sb[:, i * WBLK + k * 128 : i * WBLK + (k + 1) * 128],
                            xt_sb[:, k * W : (k + 1) * W],
                            start=(k == 0),
                            stop=(k == KT - 1),
                        )
                    act = spool.tile([128, WMAX], fp32, name="act", tag="act")
                    nc.scalar.activation(act[:, :W], pg[:, :W], silu_fn)
                    g = gpool.tile([128, WMAX], bf16, name=f"g{i}", tag=f"g{i}")
                    nc.vector.tensor_tensor(g[:, :W], act[:, :W], pu[:, :W], mult)
                    gated.append(g)
                return gated

            def emit_matmul2(o, W, gated, last=False):
                # Down-proj, token-moving: out_t[h-tile, tokens] accumulated
                # over i (cost ∝ W — no m-tile quantization), then scaled by
                # the per-token (per-column) combine weight and DMA'd out,
                # one DMA per strip, alternating the two HWDGE queues.
                for h in range(HT):
                    po = pop.tile([128, 512], fp32, name="po", tag="po")
                    for i in range(IT):
                        nc.tensor.matmul(
                            po[:, :W],
                            wd_sb[:, (h * IT + i) * 128 : (h * IT + i + 1) * 128],
                            gated[i][:, :W],
                            start=(i == 0),
                            stop=(i == IT - 1),
                        )
                    ob = opool.tile([128, WMAX], bf16, name="ob", tag="ob")
                    if last and h == HT - 1:
                        # Final strip: pipeline the (PSUM-read-bound, ~600ns)
                        # scale with the out-DMA in partition halves so the
                        # last DMA starts ~0.7us earlier. Exec ends at
                        # last-DMA + the fixed ~8us semaphore-sweep epilogue,
                        # so this comes straight off the total.
                        for c, r0 in enumerate((0, 64)):
                            nc.vector.tensor_tensor(
                                ob[r0 : r0 + 64, :W],
                                po[r0 : r0 + 64, :W],
                                cwb_sb[r0 : r0 + 64, o : o + W],
                                mult,
                            )
                            eng = nc.sync if c == 0 else nc.scalar
                            eng.dma_start(
                                out_t.ap()[h * 128 + r0 : h * 128 + r0 + 64, o : o + W],
                                ob[r0 : r0 + 64, :W],
                            )
                    else:
                        # One DMA per strip (transfers stripe across the 16 HW
                        # engines regardless): fewer DMAs = fewer completion
                        # semaphores on the critical epilogue path.
                        nc.vector.tensor_tensor(
                            ob[:, :W], po[:, :W], cwb_sb[:, o : o + W], mult
                        )
                        eng = nc.sync if h % 2 == 0 else nc.scalar
                        eng.dma_start(
                            out_t.ap()[h * 128 : (h + 1) * 128, o : o + W],
                            ob[:, :W],
                        )

            # Window pipeline: matmul2 of window t is emitted after matmul1 of
            # window t+1 (gpool bufs=2 keeps both windows' gated tiles live),
            # so the PE never waits on the scalar/vector gated production.
            pending = None
            for wi, (o, W) in enumerate(windows):
                # Window-1 prefetch rides the sync queue behind the weight
                # stream (issue-paced, can't flood startup); windows 2+ go on
                # gpsimd where the xpool buffer hazard gates them until the
                # window two before has been consumed.
                if wi == 0:
                    xt_sb = xt0_sb
                else:
                    eng = nc.sync if wi == 1 else nc.gpsimd
                    xt_sb = dma_xt_window(o, W, ksplits=[8, 8],
                                          engines=[eng] * 2)
                gated = emit_matmul1(xt_sb, W, fillers=(wi == 0))
                if pending is not None:
                    emit_matmul2(*pending)
                pending = (o, W, gated)
            emit_matmul2(*pending, last=True)

    nc.compile()
    return nc


def kernel(
    hidden_states: np.ndarray,
    gate_w: np.ndarray,
    w_gate: np.ndarray,
    w_up: np.ndarray,
    w_down: np.ndarray,
) -> np.ndarray:
    from concourse.bass_utils import run_bass_kernel_spmd

    x = np.asarray(hidden_states, dtype=np.float32).reshape(-1, H)
    gate_w = np.asarray(gate_w, dtype=np.float32)
    w_gate = np.asarray(w_gate, dtype=np.float32)
    w_up = np.asarray(w_up, dtype=np.float32)
    w_down = np.asarray(w_down, dtype=np.float32)
    T = x.shape[0]

    # Router (the sharding decision): softmax over experts, top-2, renormalize.
    logits = x @ gate_w.T
    logits -= logits.max(axis=-1, keepdims=True)
    ex = np.exp(logits)
    probs = ex / ex.sum(axis=-1, keepdims=True)
    topk_i = np.argpartition(-probs, K - 1, axis=-1)[:, :K]  # [T, K]
    topk_w = np.take_along_axis(probs, topk_i, axis=-1)
    denom = topk_w.sum(axis=-1)  # [T]

    sels, cws = [], []
    for e in range(E):
        sel = np.nonzero((topk_i == e).any(axis=1))[0]
        sels.append(sel)
        cws.append(probs[sel, e] / denom[sel])

    max_count = max(len(s) for s in sels)
    C = max(128, max_count)
    windows = _windows(C)

    if C not in _NC_CACHE:
        _NC_CACHE[C] = _build_nc(C)
    nc = _NC_CACHE[C]

    # Dispatch: gather each expert's tokens (transposed, bf16) + weights,
    # swizzled into the SBUF-image layouts the kernel's DMAs expect.
    xt_full = np.ascontiguousarray(x.T.astype(ml_dtypes.bfloat16))  # [H, T]

    def swz_w(w):  # [H, I] -> [128, IT*KT*128] i-block-major image
        return np.ascontiguousarray(
            w.astype(ml_dtypes.bfloat16)
            .reshape(KT, 128, IT, 128)
            .transpose(1, 2, 0, 3)
            .reshape(128, IT * KT * 128)
        )

    def swz_wd(w):  # [I, H] -> [128, HT*IT*128] h-major image
        return np.ascontiguousarray(
            w.astype(ml_dtypes.bfloat16)
            .reshape(IT, 128, HT, 128)
            .transpose(1, 2, 0, 3)
            .reshape(128, HT * IT * 128)
        )

    def swz_xt(xpad):  # [H, C] -> [128, KT*C] window-major image
        blocks = [
            xpad[:, o : o + W].reshape(KT, 128, W).transpose(1, 0, 2).reshape(128, -1)
            for o, W in windows
        ]
        return np.ascontiguousarray(np.concatenate(blocks, axis=1))

    in_maps = []
    for e in range(E):
        sel = sels[e]
        xpad = np.zeros((H, C), dtype=ml_dtypes.bfloat16)
        xpad[:, : len(sel)] = xt_full[:, sel]
        cw_flat = np.zeros(C, dtype=np.float32)
        cw_flat[: len(sel)] = cws[e]
        cwb = np.ascontiguousarray(np.broadcast_to(cw_flat, (128, C)))
        in_maps.append(
            {
                "xt": swz_xt(xpad),
                "wg": swz_w(w_gate[e]),
                "wu": swz_w(w_up[e]),
                "wd": swz_wd(w_down[e]),
                "cwb": cwb,
            }
        )

    trace = bool(os.environ.get("BASS_MOE_TRACE"))
    res = run_bass_kernel_spmd(
        nc, in_maps, core_ids=list(range(N_CORES)), trace=trace
    )
    if trace and res.exec_time_ns is not None:
        print(f"HW exec time: {res.exec_time_ns} ns")

    # Combine: scatter-add each expert's (already weight-scaled) rows.
    out_full = np.zeros((T, H), dtype=np.float32)
    for e in range(E):
        sel = sels[e]
        out_full[sel] += res.results[e]["out_t"][:, : len(sel)].T.astype(np.float32)
    return out_full.reshape(B, S, H)
